# revision 4
# baseline (speedup 1.0000x reference)
"""Trainium2 Bass kernel for nn_DirectionalAlignment.

Computation (per batch b, plane p):
    scores = x @ x.T / sqrt(D)            # (S, S)
    red    = sum(scores * w[p], axis=-1)  # (S, 1)
    y      = x + red
    out    = BatchNorm2d(y)               # per-plane stats over (B, S, D)

Rewritten to avoid materializing scores:
    u   = (w[p].T/sqrt(D)).T @ x = (w[p]/sqrt(D)) @ x   # (S, D) matmul, f32r on PE
    red = rowsum(x * u)                                 # fused on DVE (scalar_tensor_tensor)
    y   = x + red                                       # in-place tensor_scalar (+ rowsum accum)
    stats: sum(y) from TS accum, sum(y^2) from ACT Square accum,
           cross-partition totals via gpsimd partition_all_reduce
    out = y * (gamma*istd) + (beta - mean*gamma*istd)   # one tensor_scalar per plane

Sharding: planes (P=64) split across 8 cores, 8 planes each. BN stats are
per-plane so no collectives are needed. Weights are passed pre-transposed
and pre-scaled by 1/sqrt(D) from the host.
"""

import numpy as np
from contextlib import ExitStack

B, P, S, D = 8, 64, 256, 256
N_CORES = 8
PPC = P // N_CORES  # planes per core
BN_EPS = 1e-5
NTOT = B * S * D  # elements per plane for BN stats

_CACHE = {}


def _build_nc(exact=True, pass2_pool=True, reps=1):
    import concourse.tile as tile
    from concourse import bacc, mybir, bass_isa

    F32 = mybir.dt.float32
    F32R = mybir.dt.float32r
    Alu = mybir.AluOpType
    Act = mybir.ActivationFunctionType

    nc = bacc.Bacc("TRN2", target_bir_lowering=False, debug=False,
                   enable_asserts=False)
    x_d = nc.dram_tensor("x", [B, PPC, S, D], F32, kind="ExternalInput").ap()
    wt_d = nc.dram_tensor("wt", [PPC, S, S], F32, kind="ExternalInput").ap()
    g_d = nc.dram_tensor("gamma", [1, PPC], F32, kind="ExternalInput").ap()
    be_d = nc.dram_tensor("beta", [1, PPC], F32, kind="ExternalInput").ap()
    o_d = nc.dram_tensor("out", [B, PPC, S, D], F32, kind="ExternalOutput").ap()

    with tile.TileContext(nc) as tc, ExitStack() as ctx:
        xpool = ctx.enter_context(tc.tile_pool(name="x", bufs=PPC))
        wpool = ctx.enter_context(tc.tile_pool(name="w", bufs=PPC))
        xrpool = ctx.enter_context(tc.tile_pool(name="xr", bufs=3))
        upool = ctx.enter_context(tc.tile_pool(name="u", bufs=2, space="PSUM"))
        prodpool = ctx.enter_context(tc.tile_pool(name="prod", bufs=2))
        sqpool = ctx.enter_context(tc.tile_pool(name="sqs", bufs=2))
        redpool = ctx.enter_context(tc.tile_pool(name="red", bufs=4))
        sumpool = ctx.enter_context(tc.tile_pool(name="sums", bufs=3))
        tiny = ctx.enter_context(tc.tile_pool(name="tiny", bufs=24))
        const = ctx.enter_context(tc.tile_pool(name="const", bufs=1))

        # gamma/beta broadcast to all partitions (once)
        g1 = const.tile([1, PPC], F32)
        b1 = const.tile([1, PPC], F32)
        gb = const.tile([128, PPC], F32)
        bb = const.tile([128, PPC], F32)
        nc.sync.dma_start(g1[:], g_d[:])
        nc.sync.dma_start(b1[:], be_d[:])
        nc.gpsimd.partition_broadcast(gb[:], g1[:], channels=128)
        nc.gpsimd.partition_broadcast(bb[:], b1[:], channels=128)

        def body():
            for p in range(PPC):
                # ---- load x (exact f32) and wT (f32r) ----
                xt = xpool.tile([128, 2, B, D], F32)
                for tc in range(2):
                    nc.sync.dma_start(
                        xt[:, tc],
                        x_d[:, p, tc * 128:(tc + 1) * 128, :].rearrange(
                            "b r d -> r b d"))
                wt = wpool.tile([128, 2, S], F32R)
                nc.sync.dma_start(
                    wt[:],
                    wt_d[p].rearrange("(tc r) s -> r tc s", tc=2).bitcast(F32R))

                # ---- f32r copy of x for the matmul (rhs) ----
                if exact:
                    xr = [xrpool.tile([128, B, D], F32R, name=f"xr{p}_{k}",
                                      tag="xr") for k in range(2)]
                    for k in range(2):
                        nc.scalar.copy(xr[k][:], xt[:, k])
                    rhs = lambda k, j: xr[k][:, 2 * j:2 * j + 2, :]
                else:
                    # x itself is DMA'd rounded; reuse as rhs
                    rhs = lambda k, j: xt[:, k, 2 * j:2 * j + 2, :].bitcast(F32R)

                sums = sumpool.tile([128, 18], F32)  # 16 x sum(y), 2 x sum(y^2)

                for m in range(2):  # output row-chunk of u (s axis)
                    u_ps = upool.tile([128, B, D], F32)
                    for k in range(2):  # contraction chunk (t axis)
                        for j in range(4):  # pairs of batches
                            nc.tensor.matmul(
                                u_ps[:, 2 * j:2 * j + 2, :],
                                wt[:, k, m * 128:(m + 1) * 128],
                                rhs(k, j),
                                start=(k == 0), stop=(k == 1))

                    # red[s,b] = sum_d x*u   (prod is scratch)
                    red = redpool.tile([128, B], F32)
                    for b in range(B):
                        prod = prodpool.tile([128, D], F32)
                        nc.vector.scalar_tensor_tensor(
                            out=prod[:], in0=xt[:, m, b], scalar=1.0,
                            in1=u_ps[:, b], op0=Alu.mult, op1=Alu.mult,
                            accum_out=red[:, b:b + 1])
                    # y = x + red (in place), accum -> sum(y)
                    for b in range(B):
                        nc.vector.tensor_scalar(
                            out=xt[:, m, b], in0=xt[:, m, b],
                            scalar1=red[:, b:b + 1], scalar2=0.0,
                            op0=Alu.add, op1=Alu.add,
                            accum_out=sums[:, m * B + b:m * B + b + 1])
                    # sum(y^2) over the whole (m) region
                    sqs = sqpool.tile([128, B, D], F32)
                    nc.scalar.activation(sqs[:], xt[:, m], Act.Square,
                                         accum_out=sums[:, 16 + m:17 + m])

                # ---- finalize plane stats ----
                st = tiny.tile([128, 2], F32)
                nc.vector.tensor_reduce(st[:, 0:1], sums[:, 0:16],
                                        axis=mybir.AxisListType.X, op=Alu.add)
                nc.vector.tensor_reduce(st[:, 1:2], sums[:, 16:18],
                                        axis=mybir.AxisListType.X, op=Alu.add)
                tot = tiny.tile([128, 2], F32)
                nc.gpsimd.partition_all_reduce(tot[:], st[:], channels=128,
                                               reduce_op=bass_isa.ReduceOp.add)
                mean = tiny.tile([128, 1], F32)
                msq = tiny.tile([128, 1], F32)
                nc.vector.tensor_scalar_mul(mean[:], tot[:, 0:1], 1.0 / NTOT)
                nc.vector.tensor_scalar_mul(msq[:], tot[:, 1:2], 1.0 / NTOT)
                m2 = tiny.tile([128, 1], F32)
                nc.vector.tensor_tensor(m2[:], mean[:], mean[:], op=Alu.mult)
                vps = tiny.tile([128, 1], F32)
                nc.vector.scalar_tensor_tensor(
                    out=vps[:], in0=msq[:], scalar=BN_EPS, in1=m2[:],
                    op0=Alu.add, op1=Alu.subtract)
                rcp = tiny.tile([128, 1], F32)
                nc.vector.reciprocal(rcp[:], vps[:])
                istd = tiny.tile([128, 1], F32)
                nc.scalar.activation(istd[:], rcp[:], Act.Sqrt)
                c0 = tiny.tile([128, 1], F32)
                nc.vector.tensor_tensor(c0[:], gb[:, p:p + 1], istd[:],
                                        op=Alu.mult)
                nmc = tiny.tile([128, 1], F32)
                nc.vector.scalar_tensor_tensor(
                    out=nmc[:], in0=mean[:], scalar=-1.0, in1=c0[:],
                    op0=Alu.mult, op1=Alu.mult)  # -mean*c0
                c1 = tiny.tile([128, 1], F32)
                nc.vector.tensor_tensor(c1[:], bb[:, p:p + 1], nmc[:],
                                        op=Alu.add)

                # ---- pass2: out = y*c0 + c1, in place, then store ----
                eng = nc.gpsimd if pass2_pool else nc.vector
                eng.tensor_scalar(out=xt[:], in0=xt[:], scalar1=c0[:],
                                  scalar2=c1[:], op0=Alu.mult, op1=Alu.add)
                for tc in range(2):
                    nc.sync.dma_start(
                        o_d[:, p, tc * 128:(tc + 1) * 128, :].rearrange(
                            "b r d -> r b d"), xt[:, tc])

        if reps == 1:
            body()
        else:
            with tc.For_i(0, reps, 1):
                body()

    nc.compile()
    return nc


def _get_nc(**kw):
    key = tuple(sorted(kw.items()))
    if key not in _CACHE:
        _CACHE[key] = _build_nc(**kw)
    return _CACHE[key]


def _make_in_maps(x, weights, gamma, beta):
    inv = np.float32(1.0 / np.sqrt(D))
    wt = np.ascontiguousarray(weights.transpose(0, 2, 1)) * inv
    in_maps = []
    for c in range(N_CORES):
        sl = slice(c * PPC, (c + 1) * PPC)
        in_maps.append({
            "x": np.ascontiguousarray(x[:, sl]),
            "wt": np.ascontiguousarray(wt[sl]),
            "gamma": np.ascontiguousarray(gamma[sl]).reshape(1, PPC),
            "beta": np.ascontiguousarray(beta[sl]).reshape(1, PPC),
        })
    return in_maps


def kernel(x, weights, gamma, beta):
    from concourse.bass_utils import run_bass_kernel_spmd
    x = np.asarray(x, dtype=np.float32)
    weights = np.asarray(weights, dtype=np.float32)
    gamma = np.asarray(gamma, dtype=np.float32)
    beta = np.asarray(beta, dtype=np.float32)

    nc = _get_nc(exact=True, pass2_pool=True)
    in_maps = _make_in_maps(x, weights, gamma, beta)
    res = run_bass_kernel_spmd(nc, in_maps, core_ids=list(range(N_CORES)))
    out = np.concatenate([res.results[c]["out"] for c in range(N_CORES)],
                         axis=1)
    return out


# revision 13
# speedup vs baseline: 51504.3261x; 51504.3261x over previous
"""Trainium2 Bass kernel for nn_DirectionalAlignment.

Computation (per batch b, plane p), with x: (B, P, S, D), w: (P, S, S):
    scores = x @ x.T / sqrt(D)            # (S, S)
    red    = sum(scores * w[p], axis=-1)  # (S, 1)
    y      = x + red
    out    = BatchNorm2d(y)               # per-plane stats over (B, S, D)

Rewritten to avoid materializing scores:
    u   = (w[p]/sqrt(D)) @ x              # (S, D) matmul, f32r on the PE
    red = rowsum(x * u)                   # fused DVE scalar_tensor_tensor
    y   = x + red                         # in-place (+ rowsum accum for BN)
    stats: sum(y) via tensor_scalar accum, sum(y^2) via ACT Square accum,
           per-plane totals via gpsimd partition_all_reduce
    out = y * (gamma*istd) + (beta - mean*gamma*istd)

Sharding: planes (P=64) split across 8 cores, 8 planes each — BN stats are
per-plane so no collectives are needed.  The host pre-transposes each
core's x slice to a layout where every SBUF tile load is one large
contiguous DMA (the kernel is HBM/DMA-bound):
    xh[r, p, tc, b, d] = x[b, plane, tc*128 + r, d]   (128, PPC*2*B*D)
Weights are pre-transposed and pre-scaled by 1/sqrt(D):
    wh[r, p, tc, s] = w[plane, s, tc*128 + r] / sqrt(D)
The output leaves the device in the same transposed layout and the host
inverse-transposes when gathering.
"""

import numpy as np
from contextlib import ExitStack

B, P, S, D = 8, 64, 256, 256
N_CORES = 8
PPC = P // N_CORES  # planes per core
BN_EPS = 1e-5
NTOT = B * S * D  # elements per plane for BN stats
XFREE = PPC * 2 * B * D  # 32768 floats per partition

_CACHE = {}


def _build_nc(reps=1, out_act_q=True, cast_eng="act",
              yadd_mode="bmix", yadd_act=0, pass2_pool=False, tiny_act=True):
    import concourse.tile as tile
    from concourse import bacc, mybir, bass_isa

    F32 = mybir.dt.float32
    F32R = mybir.dt.float32r
    Alu = mybir.AluOpType
    Act = mybir.ActivationFunctionType

    nc = bacc.Bacc("TRN2", target_bir_lowering=False, debug=False,
                   enable_asserts=False)
    x_d = nc.dram_tensor("x", [128, XFREE], F32, kind="ExternalInput").ap()
    wt_d = nc.dram_tensor("wt", [128, PPC * 2 * S], F32,
                          kind="ExternalInput").ap()
    g_d = nc.dram_tensor("gamma", [1, PPC], F32, kind="ExternalInput").ap()
    be_d = nc.dram_tensor("beta", [1, PPC], F32, kind="ExternalInput").ap()
    o_d = nc.dram_tensor("out", [128, XFREE], F32, kind="ExternalOutput").ap()

    with tile.TileContext(nc) as tc, ExitStack() as ctx:
        xpool = ctx.enter_context(tc.tile_pool(name="xp", bufs=PPC))
        wpool = ctx.enter_context(tc.tile_pool(name="wp", bufs=1))
        xrpool = ctx.enter_context(tc.tile_pool(name="xr", bufs=4))
        upool = ctx.enter_context(tc.tile_pool(name="u", bufs=2, space="PSUM"))
        prodpool = ctx.enter_context(tc.tile_pool(name="prod", bufs=4))
        sqpool = ctx.enter_context(tc.tile_pool(name="sqs", bufs=2))
        redpool = ctx.enter_context(tc.tile_pool(name="red", bufs=6))
        sumpool = ctx.enter_context(tc.tile_pool(name="sums", bufs=4))
        tiny = ctx.enter_context(tc.tile_pool(name="tiny", bufs=24))
        const = ctx.enter_context(tc.tile_pool(name="const", bufs=1))

        # gamma/beta broadcast to all partitions (once, outside the loop)
        g1 = const.tile([1, PPC], F32)
        b1 = const.tile([1, PPC], F32)
        gb = const.tile([128, PPC], F32)
        bb = const.tile([128, PPC], F32)
        nc.sync.dma_start(g1[:], g_d[:])
        nc.sync.dma_start(b1[:], be_d[:])
        nc.gpsimd.partition_broadcast(gb[:], g1[:], channels=128)
        nc.gpsimd.partition_broadcast(bb[:], b1[:], channels=128)

        def body():
            # per-plane tiles, each loaded/stored as one contiguous 2MB DMA
            wall = wpool.tile([128, PPC, 2, S], F32R)
            nc.sync.dma_start(
                wall[:].rearrange("r p t s -> r (p t s)"),
                wt_d[:].bitcast(F32R))
            PFREE = 2 * B * D
            oeng = nc.scalar if out_act_q else nc.sync

            for p in range(PPC):
                xt = xpool.tile([128, 2, B, D], F32, name=f"xt{p}", tag="xt")
                nc.sync.dma_start(
                    xt[:].rearrange("r t b d -> r (t b d)"),
                    x_d[:, p * PFREE:(p + 1) * PFREE])
                wt = wall[:, p]        # (128, 2, S)

                # f32r copy of x for the matmul rhs
                xr = [xrpool.tile([128, B, D], F32R, name=f"xr{p}_{k}",
                                  tag="xr") for k in range(2)]
                for k in range(2):
                    if cast_eng == "act":
                        nc.scalar.copy(xr[k][:], xt[:, k])
                    elif cast_eng == "pool":
                        nc.gpsimd.tensor_copy(xr[k][:], xt[:, k])
                    else:
                        nc.vector.tensor_copy(xr[k][:], xt[:, k])

                # cols 0:2 sum(y) batched, 2:4 sum(y^2), 4:20 sum(y) perb
                ns = 4 if yadd_mode == "batched" else 20
                sums = sumpool.tile([128, ns], F32, name=f"sums{p}", tag="sums")
                if yadd_mode == "bmix":
                    nc.vector.memset(sums[:, 5:12], 0)

                for m in range(2):  # output row-chunk of u (s axis)
                    u_ps = upool.tile([128, B, D], F32)
                    for k in range(2):  # contraction chunk (t axis)
                        for j in range(4):  # pairs of batches
                            nc.tensor.matmul(
                                u_ps[:, 2 * j:2 * j + 2, :],
                                wt[:, k, m * 128:(m + 1) * 128],
                                xr[k][:, 2 * j:2 * j + 2, :],
                                start=(k == 0), stop=(k == 1))

                    # red[s,b] = sum_d x*u   (prod is scratch)
                    red = redpool.tile([128, B], F32)
                    for b in range(B):
                        prod = prodpool.tile([128, D], F32)
                        nc.vector.scalar_tensor_tensor(
                            out=prod[:], in0=xt[:, m, b], scalar=1.0,
                            in1=u_ps[:, b], op0=Alu.mult, op1=Alu.mult,
                            accum_out=red[:, b:b + 1])

                    # y = x + red (in place), accum -> sum(y)
                    if yadd_mode == "bmix" and m == 0:
                        red_b = red[:].unsqueeze(2).broadcast_to((128, B, D))
                        nc.vector.scalar_tensor_tensor(
                            out=xt[:, m], in0=xt[:, m], scalar=0.0,
                            in1=red_b, op0=Alu.add, op1=Alu.add,
                            accum_out=sums[:, 4:5])
                    elif yadd_mode == "bmix" and m == 1:
                        for b in range(B):
                            nc.scalar.activation(
                                xt[:, m, b], xt[:, m, b], Act.Identity,
                                bias=red[:, b:b + 1], scale=1.0,
                                accum_out=sums[:, 4 + m * B + b:5 + m * B + b])
                    elif yadd_mode in ("batched", "bmix"):
                        red_b = red[:].unsqueeze(2).broadcast_to((128, B, D))
                        nc.vector.scalar_tensor_tensor(
                            out=xt[:, m], in0=xt[:, m], scalar=0.0,
                            in1=red_b, op0=Alu.add, op1=Alu.add,
                            accum_out=sums[:, m:m + 1])
                    else:
                        for b in range(B):
                            acc = sums[:, 4 + m * B + b:5 + m * B + b]
                            if b >= B - yadd_act:
                                nc.scalar.activation(
                                    xt[:, m, b], xt[:, m, b], Act.Identity,
                                    bias=red[:, b:b + 1], scale=1.0,
                                    accum_out=acc)
                            else:
                                nc.vector.tensor_scalar(
                                    out=xt[:, m, b], in0=xt[:, m, b],
                                    scalar1=red[:, b:b + 1], scalar2=0.0,
                                    op0=Alu.add, op1=Alu.add,
                                    accum_out=acc)

                    # sum(y^2) over the whole (m) region
                    sqs = sqpool.tile([128, B, D], F32)
                    nc.scalar.activation(sqs[:], xt[:, m], Act.Square,
                                         accum_out=sums[:, 2 + m:3 + m])

                # ---- finalize plane stats ----
                st = tiny.tile([128, 2], F32)
                if yadd_mode == "batched":
                    nc.vector.tensor_reduce(st[:, 0:1], sums[:, 0:2],
                                            axis=mybir.AxisListType.X, op=Alu.add)
                elif yadd_mode == "bmix":
                    # col 4 (m0 batched) + cols 12:20 (m1 perb); 5:12 memset to 0
                    nc.vector.tensor_reduce(st[:, 0:1], sums[:, 4:20],
                                            axis=mybir.AxisListType.X, op=Alu.add)
                else:
                    nc.vector.tensor_reduce(st[:, 0:1], sums[:, 4:20],
                                            axis=mybir.AxisListType.X, op=Alu.add)
                nc.vector.tensor_reduce(st[:, 1:2], sums[:, 2:4],
                                        axis=mybir.AxisListType.X, op=Alu.add)
                tot = tiny.tile([128, 2], F32)
                nc.gpsimd.partition_all_reduce(tot[:], st[:], channels=128,
                                               reduce_op=bass_isa.ReduceOp.add)
                mean = tiny.tile([128, 1], F32)
                msq = tiny.tile([128, 1], F32)
                if tiny_act:
                    nc.scalar.mul(mean[:], tot[:, 0:1], 1.0 / NTOT)
                    nc.scalar.mul(msq[:], tot[:, 1:2], 1.0 / NTOT)
                else:
                    nc.vector.tensor_scalar_mul(mean[:], tot[:, 0:1], 1.0 / NTOT)
                    nc.vector.tensor_scalar_mul(msq[:], tot[:, 1:2], 1.0 / NTOT)
                m2 = tiny.tile([128, 1], F32)
                nc.vector.tensor_tensor(m2[:], mean[:], mean[:], op=Alu.mult)
                vps = tiny.tile([128, 1], F32)
                nc.vector.scalar_tensor_tensor(
                    out=vps[:], in0=msq[:], scalar=BN_EPS, in1=m2[:],
                    op0=Alu.add, op1=Alu.subtract)
                rcp = tiny.tile([128, 1], F32)
                nc.vector.reciprocal(rcp[:], vps[:])
                istd = tiny.tile([128, 1], F32)
                nc.scalar.activation(istd[:], rcp[:], Act.Sqrt)
                c0 = tiny.tile([128, 1], F32)
                nc.vector.tensor_tensor(c0[:], gb[:, p:p + 1], istd[:],
                                        op=Alu.mult)
                nmc = tiny.tile([128, 1], F32)
                nc.vector.scalar_tensor_tensor(
                    out=nmc[:], in0=mean[:], scalar=-1.0, in1=c0[:],
                    op0=Alu.mult, op1=Alu.mult)  # -mean*c0
                c1 = tiny.tile([128, 1], F32)
                nc.vector.tensor_tensor(c1[:], bb[:, p:p + 1], nmc[:],
                                        op=Alu.add)

                # ---- pass2: out = y*c0 + c1, in place, then store ----
                p2eng = nc.gpsimd if pass2_pool else nc.vector
                p2eng.tensor_scalar(out=xt[:], in0=xt[:], scalar1=c0[:],
                                    scalar2=c1[:], op0=Alu.mult, op1=Alu.add)
                oeng.dma_start(
                    o_d[:, p * PFREE:(p + 1) * PFREE],
                    xt[:].rearrange("r t b d -> r (t b d)"))

        if reps == 1:
            body()
        else:
            with tc.For_i(0, reps, 1):
                body()

    nc.compile()
    return nc


def _get_nc(**kw):
    key = tuple(sorted(kw.items()))
    if key not in _CACHE:
        _CACHE[key] = _build_nc(**kw)
    return _CACHE[key]


def _make_in_maps(x, weights, gamma, beta):
    inv = np.float32(1.0 / np.sqrt(D))
    # wh[r, p_all, tc, s] = w[p_all, s, tc*128+r] / sqrt(D)
    wh = weights.reshape(P, S, 2, 128).transpose(3, 0, 2, 1) * inv
    in_maps = []
    for c in range(N_CORES):
        sl = slice(c * PPC, (c + 1) * PPC)
        # xh[r, p, tc, b, d] = x[b, plane, tc*128+r, d]
        xc = x[:, sl].reshape(B, PPC, 2, 128, D)
        xh = np.ascontiguousarray(xc.transpose(3, 1, 2, 0, 4)).reshape(128, XFREE)
        in_maps.append({
            "x": xh,
            "wt": np.ascontiguousarray(wh[:, sl]).reshape(128, PPC * 2 * S),
            "gamma": np.ascontiguousarray(gamma[sl]).reshape(1, PPC),
            "beta": np.ascontiguousarray(beta[sl]).reshape(1, PPC),
        })
    return in_maps


def _gather_out(results):
    # invert: oh (128, PPC, 2, B, D) -> (B, PPC, S, D) per core, concat planes
    outs = []
    for c in range(N_CORES):
        oh = results[c]["out"].reshape(128, PPC, 2, B, D)
        oc = oh.transpose(3, 1, 2, 0, 4).reshape(B, PPC, S, D)
        outs.append(oc)
    return np.ascontiguousarray(np.concatenate(outs, axis=1))


def kernel(x, weights, gamma, beta):
    from concourse.bass_utils import run_bass_kernel_spmd
    x = np.asarray(x, dtype=np.float32)
    weights = np.asarray(weights, dtype=np.float32)
    gamma = np.asarray(gamma, dtype=np.float32)
    beta = np.asarray(beta, dtype=np.float32)

    nc = _get_nc()
    in_maps = _make_in_maps(x, weights, gamma, beta)
    res = run_bass_kernel_spmd(nc, in_maps, core_ids=list(range(N_CORES)))
    return _gather_out(res.results)
